# revision 7
# baseline (speedup 1.0000x reference)
"""ASTGNN TGC stack (2-layer time-graph-convolution) on 8 TRN2 NeuronCores.

Math per batch b (N=4000 stocks, K=16 risk factors):
    L   = M @ M^T                      [N,N] (symmetric logits)
    E   = exp(relu(L)) = max(exp(L), 1)
    d   = E @ 1                        (softmax denominators; sim = E/d row-wise)
    Y1  = X @ W1
    Z1  = relu(Y1 + (E @ Y1)/d + b1)
    Y2  = Z1 @ W2
    Z2  = (Y2 + b2) - (E @ Y2)/d

Key structure exploited:
  * L is symmetric, so the tile we need as matmul lhsT for the aggregation
    (E^T with contraction dim m on partitions) is exp(L[m,n]) - i.e. the raw
    scores tile needs NO transpose.
  * Softmax denominators come free by appending a ones-column to the V
    operand ([Y,1]); division happens after aggregation.
  * No row-max subtraction needed: relu clamps logits to [0, ~55], exp is
    safe in fp32, and division by d cancels scale exactly like the
    reference's max-subtracted softmax.

Sharding: core c = 2*b + h handles batch b, output-row half h (2000 rows).
Layer-2 aggregation needs all 4000 rows of Y2, which is computed
distributed - a tiny [2000,33] AllGather between the core pairs fixes that.

E (64MB/batch) is never materialized in HBM: scores tiles are recomputed on
the fly in both passes (the "memory" trap of this problem), optionally with
an SBUF cache for part of pass 2.
"""

import numpy as np

import concourse.bass as bass
import concourse.mybir as mybir
import concourse.tile as tile
from concourse import bacc
from concourse.bass_utils import run_bass_kernel_spmd

F32 = mybir.dt.float32

B, N, F_IN, F_HID, F_OUT, K = 4, 4000, 32, 64, 32, 16
NCORES = 8
NH = N // 2  # 2000 output rows per core

P = 128
NBLK = 512  # score-tile width (psum bank)


def _blocks(total, step):
    out = []
    o = 0
    while o < total:
        out.append((o, min(step, total - o)))
        o += step
    return out


def build_nc(n_et_cached=0, debug=False):
    """Build the SPMD Bass program (identical on all 8 cores)."""
    nc = bacc.Bacc(
        "TRN2",
        target_bir_lowering=False,
        debug=debug,
        enable_asserts=False,
        num_devices=NCORES,
    )

    # --- DRAM I/O (per-core tensors; data differs per core) ---
    mt_d = nc.dram_tensor("mt", [K, N], F32, kind="ExternalInput").ap()
    mtn_d = nc.dram_tensor("mtn", [K, NH], F32, kind="ExternalInput").ap()
    xte_d = nc.dram_tensor("xte", [F_IN + 1, N], F32, kind="ExternalInput").ap()
    xtn_d = nc.dram_tensor("xtn", [F_IN, NH], F32, kind="ExternalInput").ap()
    w1_d = nc.dram_tensor("w1", [F_IN, F_HID], F32, kind="ExternalInput").ap()
    w1e_d = nc.dram_tensor("w1e", [F_IN + 1, F_HID + 1], F32, kind="ExternalInput").ap()
    w2e_d = nc.dram_tensor("w2e", [F_HID + 1, F_OUT + 1], F32, kind="ExternalInput").ap()
    w2b_d = nc.dram_tensor("w2b", [F_HID + 1, F_OUT], F32, kind="ExternalInput").ap()
    b1c_d = nc.dram_tensor("b1c", [F_HID, 1], F32, kind="ExternalInput").ap()
    ident_d = nc.dram_tensor("ident", [P, P], F32, kind="ExternalInput").ap()
    out_d = nc.dram_tensor("out", [NH, F_OUT], F32, kind="ExternalOutput").ap()

    mblocks = _blocks(N, P)      # 31x128 + 1x32
    nblocks = _blocks(NH, NBLK)  # 3x512 + 1x464
    chunks = _blocks(NH, P)      # 15x128 + 1x80

    with tile.TileContext(nc) as tc:
        with (
            tc.tile_pool(name="static", bufs=1) as st,
            tc.tile_pool(name="etp", bufs=6) as etp,
            tc.tile_pool(name="etraw", bufs=4) as etrawp,
            tc.tile_pool(name="small", bufs=3) as sm,
            tc.tile_pool(name="cache", bufs=1) as cachep,
            tc.tile_pool(name="ps_scores", bufs=4, space="PSUM") as ps_s,
            tc.tile_pool(name="ps_acc", bufs=2, space="PSUM") as ps_a,
            tc.tile_pool(name="ps_misc", bufs=2, space="PSUM") as ps_m,
            tc.tile_pool(name="dram", bufs=1, space="DRAM") as dramp,
        ):
            # ---------- static SBUF loads ----------
            mt = st.tile([K, N], F32)
            mtn = st.tile([K, NH], F32)
            xte = st.tile([F_IN + 1, N], F32)
            xtn = st.tile([F_IN, NH], F32)
            w1 = st.tile([F_IN, F_HID], F32)
            w1e = st.tile([F_IN + 1, F_HID + 1], F32)
            w2e = st.tile([F_HID + 1, F_OUT + 1], F32)
            w2b = st.tile([F_HID + 1, F_OUT], F32)
            b1c = st.tile([F_HID, 1], F32)
            ident = st.tile([P, P], F32)
            nc.sync.dma_start(mt[:], mt_d[:])
            nc.sync.dma_start(mtn[:], mtn_d[:])
            nc.sync.dma_start(xte[:], xte_d[:])
            nc.sync.dma_start(xtn[:], xtn_d[:])
            nc.sync.dma_start(w1[:], w1_d[:])
            nc.sync.dma_start(w1e[:], w1e_d[:])
            nc.sync.dma_start(w2e[:], w2e_d[:])
            nc.sync.dma_start(w2b[:], w2b_d[:])
            nc.sync.dma_start(b1c[:], b1c_d[:])
            nc.sync.dma_start(ident[:], ident_d[:])

            ones_row = st.tile([1, F_HID], F32)
            nc.gpsimd.memset(ones_row[:], 1.0)

            # persistent per-core state
            y1t = st.tile([F_HID, NH], F32)          # (X@W1)^T, own half
            z1te = st.tile([F_HID + 1, NH], F32)     # [Z1^T; ones], own half
            nc.gpsimd.memset(z1te[F_HID : F_HID + 1, :], 1.0)
            y1p = []                                  # [Y1,1] rows, all m
            y2all = []                                # [Y2,1] rows, all m (post-gather)
            y2b = []                                  # Y2+b2, own chunks
            for mi, (m0, mw) in enumerate(mblocks):
                y1p.append(st.tile([P, F_HID + 1], F32, name=f"y1p{mi}"))
                y2all.append(st.tile([P, F_OUT + 1], F32, name=f"y2a{mi}"))
            for ki, (c0, cw) in enumerate(chunks):
                y2b.append(st.tile([P, F_OUT], F32, name=f"y2b{ki}"))

            # ---------- Y1 precompute ----------
            for mi, (m0, mw) in enumerate(mblocks):
                psy = ps_m.tile([P, F_HID + 1], F32, name="psy", tag="psm")
                nc.tensor.matmul(
                    psy[:mw], xte[:, m0 : m0 + mw], w1e[:], start=True, stop=True
                )
                nc.vector.tensor_copy(y1p[mi][:mw], psy[:mw])
            for nj, nw in nblocks:
                psyt = ps_m.tile([F_HID, NBLK], F32, name="psyt", tag="psm")
                nc.tensor.matmul(
                    psyt[:, :nw],
                    w1[:],
                    xtn[:, nj : nj + nw],
                    start=True,
                    stop=True,
                )
                nc.vector.tensor_copy(y1t[:, nj : nj + nw], psyt[:, :nw])

            # ---------- ET cache allocation (pass2 reuse) ----------
            # per nblock, cache the first (n_et_cached // len(nblocks)) mblocks
            cache_m = n_et_cached // len(nblocks)
            cache_tiles = {}
            for j, (nj, nw) in enumerate(nblocks):
                for mi in range(cache_m):
                    cache_tiles[(j, mi)] = cachep.tile(
                        [P, NBLK], F32, name=f"etc{j}_{mi}"
                    )

            def scores_to_et(j, nj, nw, mi, m0, mw, write_cache):
                """scores matmul -> exp -> max(.,1) -> SBUF E^T tile [mw, nw]."""
                pss = ps_s.tile([P, NBLK], F32, name="pss")
                nc.tensor.matmul(
                    pss[:mw, :nw],
                    mt[:, m0 : m0 + mw],
                    mtn[:, nj : nj + nw],
                    start=True,
                    stop=True,
                )
                etr = etrawp.tile([P, NBLK], F32, name="etr")
                nc.scalar.activation(
                    etr[:mw, :nw], pss[:mw, :nw], mybir.ActivationFunctionType.Exp
                )
                if write_cache:
                    et = cache_tiles[(j, mi)]
                else:
                    et = etp.tile([P, NBLK], F32, name="et")
                nc.vector.tensor_scalar_max(et[:mw, :nw], etr[:mw, :nw], 1.0)
                return et

            # ---------- pass 1 ----------
            # Software-pipelined m-loop: emit scores (PE) PRE tiles ahead of
            # the AV matmul that consumes them, so the strict-FIFO PE queue
            # never stalls waiting on the exp/max chain of its own tile.
            PRE = 3
            nm = len(mblocks)
            for j, (nj, nw) in enumerate(nblocks):
                a1t = ps_a.tile([F_HID + 1, NBLK], F32, name="a1t", tag="acc")
                ets = {}
                for mi in range(min(PRE, nm)):
                    m0, mw = mblocks[mi]
                    ets[mi] = scores_to_et(
                        j, nj, nw, mi, m0, mw, (j, mi) in cache_tiles
                    )
                for mi, (m0, mw) in enumerate(mblocks):
                    if mi + PRE < nm:
                        m0n, mwn = mblocks[mi + PRE]
                        ets[mi + PRE] = scores_to_et(
                            j, nj, nw, mi + PRE, m0n, mwn,
                            (j, mi + PRE) in cache_tiles,
                        )
                    nc.tensor.matmul(
                        a1t[:, :nw],
                        y1p[mi][:mw],
                        ets.pop(mi)[:mw, :nw],
                        start=(mi == 0),
                        stop=(mi == nm - 1),
                    )
                # epilogue: Z1^T[:, nj:nj+nw]
                rd = sm.tile([1, NBLK], F32, name="rd")
                nc.vector.reciprocal(rd[:, :nw], a1t[F_HID : F_HID + 1, :nw])
                bcp = ps_m.tile([F_HID, NBLK], F32, name="bcp", tag="psm")
                nc.tensor.matmul(
                    bcp[:, :nw], ones_row[:], rd[:, :nw], start=True, stop=True
                )
                bcs = sm.tile([F_HID, NBLK], F32, name="bcs")
                nc.scalar.activation(
                    bcs[:, :nw], bcp[:, :nw], mybir.ActivationFunctionType.Copy
                )
                tt = sm.tile([F_HID, NBLK], F32, name="tt")
                nc.vector.tensor_tensor(
                    tt[:, :nw], a1t[:F_HID, :nw], bcs[:, :nw], mybir.AluOpType.mult
                )
                uu = sm.tile([F_HID, NBLK], F32, name="uu")
                nc.vector.tensor_tensor(
                    uu[:, :nw], tt[:, :nw], y1t[:, nj : nj + nw], mybir.AluOpType.add
                )
                nc.scalar.activation(
                    z1te[:F_HID, nj : nj + nw],
                    uu[:, :nw],
                    mybir.ActivationFunctionType.Relu,
                    bias=b1c[:],
                )

            # ---------- Y2 (own half) + AllGather ----------
            cc_in = dramp.tile([NH, F_OUT + 1], F32)
            cc_out = dramp.tile([N, F_OUT + 1], F32)
            for ki, (c0, cw) in enumerate(chunks):
                psy2 = ps_m.tile([P, F_OUT + 1], F32, name="psy2", tag="psm")
                nc.tensor.matmul(
                    psy2[:cw], z1te[:, c0 : c0 + cw], w2e[:], start=True, stop=True
                )
                y2own = sm.tile([P, F_OUT + 1], F32, name="y2own")
                nc.vector.tensor_copy(y2own[:cw], psy2[:cw])
                nc.sync.dma_start(cc_in[c0 : c0 + cw, :], y2own[:cw])
                psy2b = ps_m.tile([P, F_OUT], F32, name="psy2b", tag="psm")
                nc.tensor.matmul(
                    psy2b[:cw], z1te[:, c0 : c0 + cw], w2b[:], start=True, stop=True
                )
                nc.vector.tensor_copy(y2b[ki][:cw], psy2b[:cw])
            nc.gpsimd.collective_compute(
                "AllGather",
                mybir.AluOpType.bypass,
                replica_groups=[[0, 1], [2, 3], [4, 5], [6, 7]],
                ins=[cc_in.opt()],
                outs=[cc_out.opt()],
            )
            for mi, (m0, mw) in enumerate(mblocks):
                nc.sync.dma_start(y2all[mi][:mw], cc_out[m0 : m0 + mw, :])

            # ---------- pass 2 ----------
            # Cached tiles (mi < cache_m) need no scores/exp/max; emit the
            # uncached pipeline prologue first so PE has score work queued
            # while the AllGather completes, then drain cached AVs, then the
            # pipelined uncached section.
            for j, (nj, nw) in enumerate(nblocks):
                a2t = ps_a.tile([F_OUT + 1, NBLK], F32, name="a2t", tag="acc")
                ets = {}
                for mi in range(cache_m, min(cache_m + PRE, nm)):
                    m0, mw = mblocks[mi]
                    ets[mi] = scores_to_et(j, nj, nw, mi, m0, mw, False)
                for mi, (m0, mw) in enumerate(mblocks):
                    if mi >= cache_m and mi + PRE < nm:
                        m0n, mwn = mblocks[mi + PRE]
                        ets[mi + PRE] = scores_to_et(
                            j, nj, nw, mi + PRE, m0n, mwn, False
                        )
                    if (j, mi) in cache_tiles:
                        et = cache_tiles[(j, mi)]
                    else:
                        et = ets.pop(mi)
                    nc.tensor.matmul(
                        a2t[:, :nw],
                        y2all[mi][:mw],
                        et[:mw, :nw],
                        start=(mi == 0),
                        stop=(mi == nm - 1),
                    )
                # epilogue per 128-chunk: transpose -> divide -> combine -> store
                a2s = sm.tile([F_OUT + 1, NBLK], F32, name="a2s")
                nc.vector.tensor_copy(a2s[:, :nw], a2t[:, :nw])
                for c0, cw in _blocks(nw, P):
                    g0 = nj + c0  # offset within own half
                    ki = g0 // P
                    trp = ps_m.tile([P, F_OUT + 1], F32, name="trp", tag="psm")
                    nc.tensor.transpose(
                        trp[:cw],
                        a2s[:, c0 : c0 + cw],
                        ident[: F_OUT + 1, : F_OUT + 1],
                    )
                    rd2 = sm.tile([P, 1], F32, name="rd2")
                    nc.vector.reciprocal(rd2[:cw], trp[:cw, F_OUT : F_OUT + 1])
                    vv = sm.tile([P, F_OUT], F32, name="vv")
                    nc.vector.tensor_scalar(
                        vv[:cw],
                        trp[:cw, :F_OUT],
                        rd2[:cw],
                        None,
                        op0=mybir.AluOpType.mult,
                    )
                    z2 = sm.tile([P, F_OUT], F32, name="z2")
                    nc.vector.tensor_tensor(
                        z2[:cw], y2b[ki][:cw], vv[:cw], mybir.AluOpType.subtract
                    )
                    nc.sync.dma_start(out_d[g0 : g0 + cw, :], z2[:cw])

    nc.compile()
    return nc


def _prep_inputs(X, M, W1, b1, W2, b2):
    """Host-side prep: transposes + affine-trick weight matrices."""
    X = np.ascontiguousarray(X, dtype=np.float32)
    M = np.ascontiguousarray(M, dtype=np.float32)
    W1 = np.asarray(W1, dtype=np.float32)
    b1 = np.asarray(b1, dtype=np.float32)
    W2 = np.asarray(W2, dtype=np.float32)
    b2 = np.asarray(b2, dtype=np.float32)

    MT = np.ascontiguousarray(M.transpose(0, 2, 1))  # [B,16,N]
    XT = X.transpose(0, 2, 1)  # [B,32,N]
    XTe = np.concatenate(
        [XT, np.ones((B, 1, N), np.float32)], axis=1
    )  # [B,33,N]
    XTe = np.ascontiguousarray(XTe)

    W1e = np.zeros((F_IN + 1, F_HID + 1), np.float32)
    W1e[:F_IN, :F_HID] = W1
    W1e[F_IN, F_HID] = 1.0  # ones column of [Y1,1] via the ones row of XTe

    W2e = np.zeros((F_HID + 1, F_OUT + 1), np.float32)
    W2e[:F_HID, :F_OUT] = W2
    W2e[F_HID, F_OUT] = 1.0

    W2b = np.zeros((F_HID + 1, F_OUT), np.float32)
    W2b[:F_HID] = W2
    W2b[F_HID] = b2

    b1c = np.ascontiguousarray(b1.reshape(F_HID, 1))
    ident = np.eye(P, dtype=np.float32)

    in_maps = []
    for c in range(NCORES):
        b, h = c // 2, c % 2
        n0 = h * NH
        in_maps.append(
            {
                "mt": MT[b],
                "mtn": np.ascontiguousarray(MT[b][:, n0 : n0 + NH]),
                "xte": XTe[b],
                "xtn": np.ascontiguousarray(XT[b][:, n0 : n0 + NH]),
                "w1": W1,
                "w1e": W1e,
                "w2e": W2e,
                "w2b": W2b,
                "b1c": b1c,
                "ident": ident,
            }
        )
    return in_maps


_NC_CACHE = {}


def _get_nc(n_et_cached=44):
    key = n_et_cached
    if key not in _NC_CACHE:
        _NC_CACHE[key] = build_nc(n_et_cached=n_et_cached)
    return _NC_CACHE[key]


def kernel(X, M, W1, b1, W2, b2, _trace=False, _n_et_cached=44):
    nc = _get_nc(_n_et_cached)
    in_maps = _prep_inputs(X, M, W1, b1, W2, b2)
    res = run_bass_kernel_spmd(
        nc, in_maps, core_ids=list(range(NCORES)), trace=_trace
    )
    out = np.empty((B, N, F_OUT), np.float32)
    for c in range(NCORES):
        b, h = c // 2, c % 2
        out[b, h * NH : (h + 1) * NH] = res.results[c]["out"]
    kernel.last_results = res
    return out


# revision 10
# speedup vs baseline: 1.3351x; 1.3351x over previous
"""ASTGNN TGC stack (2-layer time-graph-convolution) on 8 TRN2 NeuronCores.

Math per batch b (N=4000 stocks, K=16 risk factors):
    L   = M @ M^T                      [N,N] (symmetric logits)
    E   = exp(relu(L)) = max(exp(L), 1)
    d   = E @ 1                        (softmax denominators; sim = E/d row-wise)
    Y1  = X @ W1
    Z1  = relu(Y1 + (E @ Y1)/d + b1)
    Y2  = Z1 @ W2
    Z2  = (Y2 + b2) - (E @ Y2)/d

Key structure exploited:
  * L is symmetric, so the tile we need as matmul lhsT for the aggregation
    (E^T with contraction dim m on partitions) is exp(L[m,n]) - i.e. the raw
    scores tile needs NO transpose.
  * Softmax denominators come free by appending a ones-column to the V
    operand ([Y,1]); division happens after aggregation.
  * No row-max subtraction needed: relu clamps logits to [0, ~55], exp is
    safe in fp32, and division by d cancels scale exactly like the
    reference's max-subtracted softmax.

Sharding: core c = 2*b + h handles batch b, output-row half h (2000 rows).
Layer-2 aggregation needs all 4000 rows of Y2, which is computed
distributed - a tiny [2000,33] AllGather between the core pairs fixes that.

E (64MB/batch) is never materialized in HBM: scores tiles are recomputed on
the fly in both passes (the "memory" trap of this problem), optionally with
an SBUF cache for part of pass 2.
"""

import numpy as np

import concourse.bass as bass
import concourse.mybir as mybir
import concourse.tile as tile
from concourse import bacc
from concourse.bass_utils import run_bass_kernel_spmd

F32 = mybir.dt.float32
F32R = mybir.dt.float32r

B, N, F_IN, F_HID, F_OUT, K = 4, 4000, 32, 64, 32, 16
NCORES = 8
NH = N // 2  # 2000 output rows per core

P = 128
NBLK = 512  # score-tile width (psum bank)


def _blocks(total, step):
    out = []
    o = 0
    while o < total:
        out.append((o, min(step, total - o)))
        o += step
    return out


def build_nc(n_et_cached=0, rmode=0, debug=False):
    # rmode 0: all-fp32 exact (4 cyc/row matmuls)
    # rmode 1: fp32r (tf32) scores + AV (1 cyc/row; ~1e-3 rel err)
    # rmode 2: compensated hi/lo fp32r scores (3 accumulating matmuls, ~exact)
    #          + fp32r AV (~5e-4 rel err)
    """Build the SPMD Bass program (identical on all 8 cores)."""
    nc = bacc.Bacc(
        "TRN2",
        target_bir_lowering=False,
        debug=debug,
        enable_asserts=False,
        num_devices=NCORES,
    )

    use_r = rmode > 0
    MDT = F32R if use_r else F32   # dtype of tensors feeding the big matmuls

    # --- DRAM I/O (per-core tensors; data differs per core) ---
    if rmode == 2:
        mt_hi_d = nc.dram_tensor("mt_hi", [K, N], F32R, kind="ExternalInput").ap()
        mt_lo_d = nc.dram_tensor("mt_lo", [K, N], F32R, kind="ExternalInput").ap()
        mtn_hi_d = nc.dram_tensor("mtn_hi", [K, NH], F32R, kind="ExternalInput").ap()
        mtn_lo_d = nc.dram_tensor("mtn_lo", [K, NH], F32R, kind="ExternalInput").ap()
    else:
        mt_d = nc.dram_tensor("mt", [K, N], MDT, kind="ExternalInput").ap()
        mtn_d = nc.dram_tensor("mtn", [K, NH], MDT, kind="ExternalInput").ap()
    xte_d = nc.dram_tensor("xte", [F_IN + 1, N], F32, kind="ExternalInput").ap()
    xtn_d = nc.dram_tensor("xtn", [F_IN, NH], F32, kind="ExternalInput").ap()
    w1_d = nc.dram_tensor("w1", [F_IN, F_HID], F32, kind="ExternalInput").ap()
    w1e_d = nc.dram_tensor("w1e", [F_IN + 1, F_HID + 1], F32, kind="ExternalInput").ap()
    w2e_d = nc.dram_tensor("w2e", [F_HID + 1, F_OUT + 1], F32, kind="ExternalInput").ap()
    w2b_d = nc.dram_tensor("w2b", [F_HID + 1, F_OUT], F32, kind="ExternalInput").ap()
    b1c_d = nc.dram_tensor("b1c", [F_HID, 1], F32, kind="ExternalInput").ap()
    ident_d = nc.dram_tensor("ident", [P, P], F32, kind="ExternalInput").ap()
    out_d = nc.dram_tensor("out", [NH, F_OUT], F32, kind="ExternalOutput").ap()

    mblocks = _blocks(N, P)      # 31x128 + 1x32
    nblocks = _blocks(NH, NBLK)  # 3x512 + 1x464
    chunks = _blocks(NH, P)      # 15x128 + 1x80

    with tile.TileContext(nc) as tc:
        with (
            tc.tile_pool(name="static", bufs=1) as st,
            tc.tile_pool(name="etp", bufs=6) as etp,
            tc.tile_pool(name="etraw", bufs=4) as etrawp,
            tc.tile_pool(name="small", bufs=3) as sm,
            tc.tile_pool(name="cache", bufs=1) as cachep,
            tc.tile_pool(name="ps_scores", bufs=4, space="PSUM") as ps_s,
            tc.tile_pool(name="ps_acc", bufs=2, space="PSUM") as ps_a,
            tc.tile_pool(name="ps_misc", bufs=2, space="PSUM") as ps_m,
            tc.tile_pool(name="dram", bufs=1, space="DRAM") as dramp,
        ):
            # ---------- static SBUF loads ----------
            if rmode == 2:
                mt_hi = st.tile([K, N], F32R)
                mt_lo = st.tile([K, N], F32R)
                mtn_hi = st.tile([K, NH], F32R)
                mtn_lo = st.tile([K, NH], F32R)
            else:
                mt = st.tile([K, N], MDT)
                mtn = st.tile([K, NH], MDT)
            xte = st.tile([F_IN + 1, N], F32)
            xtn = st.tile([F_IN, NH], F32)
            w1 = st.tile([F_IN, F_HID], F32)
            w1e = st.tile([F_IN + 1, F_HID + 1], F32)
            w2e = st.tile([F_HID + 1, F_OUT + 1], F32)
            w2b = st.tile([F_HID + 1, F_OUT], F32)
            b1c = st.tile([F_HID, 1], F32)
            ident = st.tile([P, P], F32)
            if rmode == 2:
                nc.sync.dma_start(mt_hi[:], mt_hi_d[:])
                nc.sync.dma_start(mt_lo[:], mt_lo_d[:])
                nc.sync.dma_start(mtn_hi[:], mtn_hi_d[:])
                nc.sync.dma_start(mtn_lo[:], mtn_lo_d[:])
            else:
                nc.sync.dma_start(mt[:], mt_d[:])
                nc.sync.dma_start(mtn[:], mtn_d[:])
            nc.sync.dma_start(xte[:], xte_d[:])
            nc.sync.dma_start(xtn[:], xtn_d[:])
            nc.sync.dma_start(w1[:], w1_d[:])
            nc.sync.dma_start(w1e[:], w1e_d[:])
            nc.sync.dma_start(w2e[:], w2e_d[:])
            nc.sync.dma_start(w2b[:], w2b_d[:])
            nc.sync.dma_start(b1c[:], b1c_d[:])
            nc.sync.dma_start(ident[:], ident_d[:])

            ones_row = st.tile([1, F_HID], F32)
            nc.gpsimd.memset(ones_row[:], 1.0)

            # persistent per-core state
            y1t = st.tile([F_HID, NH], F32)          # (X@W1)^T, own half
            z1te = st.tile([F_HID + 1, NH], F32)     # [Z1^T; ones], own half
            nc.gpsimd.memset(z1te[F_HID : F_HID + 1, :], 1.0)
            y1p = []                                  # [Y1,1] rows, all m
            y2all = []                                # [Y2,1] rows, all m (post-gather)
            y2b = []                                  # Y2+b2, own chunks
            for mi, (m0, mw) in enumerate(mblocks):
                y1p.append(st.tile([P, F_HID + 1], MDT, name=f"y1p{mi}"))
                y2all.append(st.tile([P, F_OUT + 1], MDT, name=f"y2a{mi}"))
            for ki, (c0, cw) in enumerate(chunks):
                y2b.append(st.tile([P, F_OUT], F32, name=f"y2b{ki}"))

            # ---------- Y1 precompute ----------
            for mi, (m0, mw) in enumerate(mblocks):
                psy = ps_m.tile([P, F_HID + 1], F32, name="psy", tag="psm")
                nc.tensor.matmul(
                    psy[:mw], xte[:, m0 : m0 + mw], w1e[:], start=True, stop=True
                )
                nc.vector.tensor_copy(y1p[mi][:mw], psy[:mw])
            for nj, nw in nblocks:
                psyt = ps_m.tile([F_HID, NBLK], F32, name="psyt", tag="psm")
                nc.tensor.matmul(
                    psyt[:, :nw],
                    w1[:],
                    xtn[:, nj : nj + nw],
                    start=True,
                    stop=True,
                )
                nc.vector.tensor_copy(y1t[:, nj : nj + nw], psyt[:, :nw])

            # ---------- ET cache allocation (pass2 reuse) ----------
            # per nblock, cache the first (n_et_cached // len(nblocks)) mblocks
            cache_m = n_et_cached // len(nblocks)
            cache_tiles = {}
            for j, (nj, nw) in enumerate(nblocks):
                for mi in range(cache_m):
                    cache_tiles[(j, mi)] = cachep.tile(
                        [P, NBLK], MDT, name=f"etc{j}_{mi}"
                    )

            def scores_to_et(j, nj, nw, mi, m0, mw, write_cache):
                """scores matmul -> exp -> max(.,1) -> SBUF E^T tile [mw, nw]."""
                pss = ps_s.tile([P, NBLK], F32, name="pss")
                if rmode == 2:
                    # L = Mh Nh^T + Ml Nh^T + Mh Nl^T  (drop Ml Nl ~ 2^-22)
                    nc.tensor.matmul(
                        pss[:mw, :nw], mt_hi[:, m0 : m0 + mw],
                        mtn_hi[:, nj : nj + nw], start=True, stop=False)
                    nc.tensor.matmul(
                        pss[:mw, :nw], mt_lo[:, m0 : m0 + mw],
                        mtn_hi[:, nj : nj + nw], start=False, stop=False)
                    nc.tensor.matmul(
                        pss[:mw, :nw], mt_hi[:, m0 : m0 + mw],
                        mtn_lo[:, nj : nj + nw], start=False, stop=True)
                else:
                    nc.tensor.matmul(
                        pss[:mw, :nw], mt[:, m0 : m0 + mw],
                        mtn[:, nj : nj + nw], start=True, stop=True)
                etr = etrawp.tile([P, NBLK], F32, name="etr")
                nc.scalar.activation(
                    etr[:mw, :nw], pss[:mw, :nw], mybir.ActivationFunctionType.Exp
                )
                if write_cache:
                    et = cache_tiles[(j, mi)]
                else:
                    et = etp.tile([P, NBLK], MDT, name="et")
                nc.vector.tensor_scalar_max(et[:mw, :nw], etr[:mw, :nw], 1.0)
                return et

            # ---------- pass 1 ----------
            # Software-pipelined m-loop: emit scores (PE) PRE tiles ahead of
            # the AV matmul that consumes them, so the strict-FIFO PE queue
            # never stalls waiting on the exp/max chain of its own tile.
            PRE = 3
            nm = len(mblocks)
            for j, (nj, nw) in enumerate(nblocks):
                a1t = ps_a.tile([F_HID + 1, NBLK], F32, name="a1t", tag="acc")
                ets = {}
                for mi in range(min(PRE, nm)):
                    m0, mw = mblocks[mi]
                    ets[mi] = scores_to_et(
                        j, nj, nw, mi, m0, mw, (j, mi) in cache_tiles
                    )
                for mi, (m0, mw) in enumerate(mblocks):
                    if mi + PRE < nm:
                        m0n, mwn = mblocks[mi + PRE]
                        ets[mi + PRE] = scores_to_et(
                            j, nj, nw, mi + PRE, m0n, mwn,
                            (j, mi + PRE) in cache_tiles,
                        )
                    nc.tensor.matmul(
                        a1t[:, :nw], y1p[mi][:mw], ets.pop(mi)[:mw, :nw],
                        start=(mi == 0), stop=(mi == nm - 1),
                    )
                # epilogue: Z1^T[:, nj:nj+nw]
                rd = sm.tile([1, NBLK], F32, name="rd")
                nc.vector.reciprocal(rd[:, :nw], a1t[F_HID : F_HID + 1, :nw])
                bcp = ps_m.tile([F_HID, NBLK], F32, name="bcp", tag="psm")
                nc.tensor.matmul(
                    bcp[:, :nw], ones_row[:], rd[:, :nw], start=True, stop=True
                )
                bcs = sm.tile([F_HID, NBLK], F32, name="bcs")
                nc.scalar.activation(
                    bcs[:, :nw], bcp[:, :nw], mybir.ActivationFunctionType.Copy
                )
                tt = sm.tile([F_HID, NBLK], F32, name="tt")
                nc.vector.tensor_tensor(
                    tt[:, :nw], a1t[:F_HID, :nw], bcs[:, :nw], mybir.AluOpType.mult
                )
                uu = sm.tile([F_HID, NBLK], F32, name="uu")
                nc.vector.tensor_tensor(
                    uu[:, :nw], tt[:, :nw], y1t[:, nj : nj + nw], mybir.AluOpType.add
                )
                nc.scalar.activation(
                    z1te[:F_HID, nj : nj + nw],
                    uu[:, :nw],
                    mybir.ActivationFunctionType.Relu,
                    bias=b1c[:],
                )

            # ---------- Y2 (own half) + AllGather ----------
            cc_in = dramp.tile([NH, F_OUT + 1], MDT)
            cc_out = dramp.tile([N, F_OUT + 1], MDT)
            for ki, (c0, cw) in enumerate(chunks):
                psy2 = ps_m.tile([P, F_OUT + 1], F32, name="psy2", tag="psm")
                nc.tensor.matmul(
                    psy2[:cw], z1te[:, c0 : c0 + cw], w2e[:], start=True, stop=True
                )
                y2own = sm.tile([P, F_OUT + 1], MDT, name="y2own")
                nc.vector.tensor_copy(y2own[:cw], psy2[:cw])
                nc.sync.dma_start(cc_in[c0 : c0 + cw, :], y2own[:cw])
                psy2b = ps_m.tile([P, F_OUT], F32, name="psy2b", tag="psm")
                nc.tensor.matmul(
                    psy2b[:cw], z1te[:, c0 : c0 + cw], w2b[:], start=True, stop=True
                )
                nc.vector.tensor_copy(y2b[ki][:cw], psy2b[:cw])
            nc.gpsimd.collective_compute(
                "AllGather",
                mybir.AluOpType.bypass,
                replica_groups=[[0, 1], [2, 3], [4, 5], [6, 7]],
                ins=[cc_in.opt()],
                outs=[cc_out.opt()],
            )
            for mi, (m0, mw) in enumerate(mblocks):
                nc.sync.dma_start(y2all[mi][:mw], cc_out[m0 : m0 + mw, :])

            # ---------- pass 2 ----------
            # Cached tiles (mi < cache_m) need no scores/exp/max; emit the
            # uncached pipeline prologue first so PE has score work queued
            # while the AllGather completes, then drain cached AVs, then the
            # pipelined uncached section.
            for j, (nj, nw) in enumerate(nblocks):
                a2t = ps_a.tile([F_OUT + 1, NBLK], F32, name="a2t", tag="acc")
                ets = {}
                for mi in range(cache_m, min(cache_m + PRE, nm)):
                    m0, mw = mblocks[mi]
                    ets[mi] = scores_to_et(j, nj, nw, mi, m0, mw, False)
                for mi, (m0, mw) in enumerate(mblocks):
                    if mi >= cache_m and mi + PRE < nm:
                        m0n, mwn = mblocks[mi + PRE]
                        ets[mi + PRE] = scores_to_et(
                            j, nj, nw, mi + PRE, m0n, mwn, False
                        )
                    if (j, mi) in cache_tiles:
                        et = cache_tiles[(j, mi)]
                    else:
                        et = ets.pop(mi)
                    nc.tensor.matmul(
                        a2t[:, :nw], y2all[mi][:mw], et[:mw, :nw],
                        start=(mi == 0), stop=(mi == nm - 1),
                    )
                # epilogue per 128-chunk: transpose -> divide -> combine -> store
                a2s = sm.tile([F_OUT + 1, NBLK], F32, name="a2s")
                nc.vector.tensor_copy(a2s[:, :nw], a2t[:, :nw])
                for c0, cw in _blocks(nw, P):
                    g0 = nj + c0  # offset within own half
                    ki = g0 // P
                    trp = ps_m.tile([P, F_OUT + 1], F32, name="trp", tag="psm")
                    nc.tensor.transpose(
                        trp[:cw],
                        a2s[:, c0 : c0 + cw],
                        ident[: F_OUT + 1, : F_OUT + 1],
                    )
                    rd2 = sm.tile([P, 1], F32, name="rd2")
                    nc.vector.reciprocal(rd2[:cw], trp[:cw, F_OUT : F_OUT + 1])
                    vv = sm.tile([P, F_OUT], F32, name="vv")
                    nc.vector.tensor_scalar(
                        vv[:cw],
                        trp[:cw, :F_OUT],
                        rd2[:cw],
                        None,
                        op0=mybir.AluOpType.mult,
                    )
                    z2 = sm.tile([P, F_OUT], F32, name="z2")
                    nc.vector.tensor_tensor(
                        z2[:cw], y2b[ki][:cw], vv[:cw], mybir.AluOpType.subtract
                    )
                    nc.sync.dma_start(out_d[g0 : g0 + cw, :], z2[:cw])

    nc.compile()
    return nc


def _tf32_round(x):
    xi = x.astype(np.float32).view(np.uint32)
    xi = (xi + np.uint32(0x1000)) & np.uint32(0xFFFFE000)
    return xi.view(np.float32)


def _prep_inputs(X, M, W1, b1, W2, b2, rmode=0):
    """Host-side prep: transposes + affine-trick weight matrices."""
    X = np.ascontiguousarray(X, dtype=np.float32)
    M = np.ascontiguousarray(M, dtype=np.float32)
    W1 = np.asarray(W1, dtype=np.float32)
    b1 = np.asarray(b1, dtype=np.float32)
    W2 = np.asarray(W2, dtype=np.float32)
    b2 = np.asarray(b2, dtype=np.float32)

    MT = np.ascontiguousarray(M.transpose(0, 2, 1))  # [B,16,N]
    XT = X.transpose(0, 2, 1)  # [B,32,N]
    XTe = np.concatenate(
        [XT, np.ones((B, 1, N), np.float32)], axis=1
    )  # [B,33,N]
    XTe = np.ascontiguousarray(XTe)

    W1e = np.zeros((F_IN + 1, F_HID + 1), np.float32)
    W1e[:F_IN, :F_HID] = W1
    W1e[F_IN, F_HID] = 1.0  # ones column of [Y1,1] via the ones row of XTe

    W2e = np.zeros((F_HID + 1, F_OUT + 1), np.float32)
    W2e[:F_HID, :F_OUT] = W2
    W2e[F_HID, F_OUT] = 1.0

    W2b = np.zeros((F_HID + 1, F_OUT), np.float32)
    W2b[:F_HID] = W2
    W2b[F_HID] = b2

    b1c = np.ascontiguousarray(b1.reshape(F_HID, 1))
    ident = np.eye(P, dtype=np.float32)

    if rmode == 2:
        MT_hi = _tf32_round(MT)
        MT_lo = _tf32_round(MT - MT_hi)

    in_maps = []
    for c in range(NCORES):
        b, h = c // 2, c % 2
        n0 = h * NH
        m = {
            "xte": XTe[b],
            "xtn": np.ascontiguousarray(XT[b][:, n0 : n0 + NH]),
            "w1": W1,
            "w1e": W1e,
            "w2e": W2e,
            "w2b": W2b,
            "b1c": b1c,
            "ident": ident,
        }
        if rmode == 2:
            m["mt_hi"] = MT_hi[b]
            m["mt_lo"] = MT_lo[b]
            m["mtn_hi"] = np.ascontiguousarray(MT_hi[b][:, n0 : n0 + NH])
            m["mtn_lo"] = np.ascontiguousarray(MT_lo[b][:, n0 : n0 + NH])
        else:
            m["mt"] = MT[b]
            m["mtn"] = np.ascontiguousarray(MT[b][:, n0 : n0 + NH])
        in_maps.append(m)
    return in_maps


_NC_CACHE = {}


def _get_nc(n_et_cached=36, rmode=0):
    key = (n_et_cached, rmode)
    if key not in _NC_CACHE:
        _NC_CACHE[key] = build_nc(n_et_cached=n_et_cached, rmode=rmode)
    return _NC_CACHE[key]


def kernel(X, M, W1, b1, W2, b2, _trace=False, _n_et_cached=36, _rmode=0):
    nc = _get_nc(_n_et_cached, _rmode)
    in_maps = _prep_inputs(X, M, W1, b1, W2, b2, rmode=_rmode)
    res = run_bass_kernel_spmd(
        nc, in_maps, core_ids=list(range(NCORES)), trace=_trace
    )
    out = np.empty((B, N, F_OUT), np.float32)
    for c in range(NCORES):
        b, h = c // 2, c % 2
        out[b, h * NH : (h + 1) * NH] = res.results[c]["out"]
    kernel.last_results = res
    return out


# revision 13
# speedup vs baseline: 2.1396x; 1.6026x over previous
"""ASTGNN TGC stack (2-layer time-graph-convolution) on 8 TRN2 NeuronCores.

Math per batch b (N=4000 stocks, K=16 risk factors):
    L   = M @ M^T                      [N,N] (symmetric logits)
    E   = exp(relu(L)) = max(exp(L), 1)
    d   = E @ 1                        (softmax denominators; sim = E/d row-wise)
    Y1  = X @ W1
    Z1  = relu(Y1 + (E @ Y1)/d + b1)
    Y2  = Z1 @ W2
    Z2  = (Y2 + b2) - (E @ Y2)/d

Key structure exploited:
  * L is symmetric, so the tile we need as matmul lhsT for the aggregation
    (E^T with contraction dim m on partitions) is exp(L[m,n]) - i.e. the raw
    scores tile needs NO transpose.
  * Softmax denominators come free by appending a ones-column to the V
    operand ([Y,1]); division happens after aggregation.
  * No row-max subtraction needed: relu clamps logits to [0, ~55], exp is
    safe in fp32, and division by d cancels scale exactly like the
    reference's max-subtracted softmax.

Sharding: core c = 2*b + h handles batch b, output-row half h (2000 rows).
Layer-2 aggregation needs all 4000 rows of Y2, which is computed
distributed - a tiny [2000,33] AllGather between the core pairs fixes that.

E (64MB/batch) is never materialized in HBM: scores tiles are recomputed on
the fly in both passes (the "memory" trap of this problem), optionally with
an SBUF cache for part of pass 2.
"""

import numpy as np

import concourse.bass as bass
import concourse.mybir as mybir
import concourse.tile as tile
from concourse import bacc
from concourse.bass_utils import run_bass_kernel_spmd

F32 = mybir.dt.float32
F32R = mybir.dt.float32r

B, N, F_IN, F_HID, F_OUT, K = 4, 4000, 32, 64, 32, 16
NCORES = 8
NH = N // 2  # 2000 output rows per core

P = 128
NBLK = 512  # score-tile width (psum bank)


def _blocks(total, step):
    out = []
    o = 0
    while o < total:
        out.append((o, min(step, total - o)))
        o += step
    return out


def build_nc(n_et_cached=0, rmode=0, debug=False):
    # rmode 0: all-fp32 exact (4 cyc/row matmuls)
    # rmode 1: fp32r (tf32) scores + AV (1 cyc/row; ~1e-3 rel err)
    # rmode 2: compensated hi/lo fp32r scores (3 accumulating matmuls, ~exact)
    #          + fp32r AV (~5e-4 rel err)
    """Build the SPMD Bass program (identical on all 8 cores)."""
    nc = bacc.Bacc(
        "TRN2",
        target_bir_lowering=False,
        debug=debug,
        enable_asserts=False,
        num_devices=NCORES,
    )

    use_r = rmode > 0
    MDT = F32R if use_r else F32   # dtype of tensors feeding the big matmuls

    # --- DRAM I/O (per-core tensors; data differs per core) ---
    if rmode == 2:
        mt_hi_d = nc.dram_tensor("mt_hi", [K, N], F32R, kind="ExternalInput").ap()
        mt_lo_d = nc.dram_tensor("mt_lo", [K, N], F32R, kind="ExternalInput").ap()
        mtn_hi_d = nc.dram_tensor("mtn_hi", [K, NH], F32R, kind="ExternalInput").ap()
        mtn_lo_d = nc.dram_tensor("mtn_lo", [K, NH], F32R, kind="ExternalInput").ap()
    else:
        mt_d = nc.dram_tensor("mt", [K, N], MDT, kind="ExternalInput").ap()
        mtn_d = nc.dram_tensor("mtn", [K, NH], MDT, kind="ExternalInput").ap()
    xte_d = nc.dram_tensor("xte", [F_IN + 1, N], F32, kind="ExternalInput").ap()
    xtn_d = nc.dram_tensor("xtn", [F_IN, NH], F32, kind="ExternalInput").ap()
    w1_d = nc.dram_tensor("w1", [F_IN, F_HID], F32, kind="ExternalInput").ap()
    w1e_d = nc.dram_tensor("w1e", [F_IN + 1, F_HID + 1], F32, kind="ExternalInput").ap()
    w2e_d = nc.dram_tensor("w2e", [F_HID + 1, F_OUT + 1], F32, kind="ExternalInput").ap()
    w2b_d = nc.dram_tensor("w2b", [F_HID + 1, F_OUT], F32, kind="ExternalInput").ap()
    b1c_d = nc.dram_tensor("b1c", [F_HID, 1], F32, kind="ExternalInput").ap()
    ident_d = nc.dram_tensor("ident", [P, P], F32, kind="ExternalInput").ap()
    out_d = nc.dram_tensor("out", [NH, F_OUT], F32, kind="ExternalOutput").ap()

    mblocks = _blocks(N, P)      # 31x128 + 1x32
    nblocks = _blocks(NH, NBLK)  # 3x512 + 1x464
    chunks = _blocks(NH, P)      # 15x128 + 1x80

    from contextlib import ExitStack

    with tile.TileContext(nc) as tc, ExitStack() as es:
        if True:
            st = es.enter_context(tc.tile_pool(name="static", bufs=1))
            ps_s = es.enter_context(tc.tile_pool(name="ps_scores", bufs=4, space="PSUM"))
            ps_a = es.enter_context(tc.tile_pool(name="ps_acc", bufs=2, space="PSUM"))
            ps_m = es.enter_context(tc.tile_pool(name="ps_misc", bufs=2, space="PSUM"))
            dramp = es.enter_context(tc.tile_pool(name="dram", bufs=1, space="DRAM"))
            # ---------- static SBUF loads ----------
            if rmode == 2:
                mt_hi = st.tile([K, N], F32R)
                mt_lo = st.tile([K, N], F32R)
                mtn_hi = st.tile([K, NH], F32R)
                mtn_lo = st.tile([K, NH], F32R)
            else:
                mt = st.tile([K, N], MDT)
                mtn = st.tile([K, NH], MDT)
            xpool_ctx = tc.tile_pool(name="xtmp", bufs=1)
            xpool = xpool_ctx.__enter__()
            xte = xpool.tile([F_IN + 1, N], F32)
            xtn = xpool.tile([F_IN, NH], F32)
            w1 = st.tile([F_IN, F_HID], F32)
            w1e = st.tile([F_IN + 1, F_HID + 1], F32)
            w2e = st.tile([F_HID + 1, F_OUT + 1], F32)
            w2b = st.tile([F_HID + 1, F_OUT], F32)
            b1c = st.tile([F_HID, 1], F32)
            ident = st.tile([P, P], F32)
            if rmode == 2:
                nc.sync.dma_start(mt_hi[:], mt_hi_d[:])
                nc.sync.dma_start(mt_lo[:], mt_lo_d[:])
                nc.sync.dma_start(mtn_hi[:], mtn_hi_d[:])
                nc.sync.dma_start(mtn_lo[:], mtn_lo_d[:])
            else:
                nc.sync.dma_start(mt[:], mt_d[:])
                nc.sync.dma_start(mtn[:], mtn_d[:])
            nc.sync.dma_start(xte[:], xte_d[:])
            nc.sync.dma_start(xtn[:], xtn_d[:])
            nc.sync.dma_start(w1[:], w1_d[:])
            nc.sync.dma_start(w1e[:], w1e_d[:])
            nc.sync.dma_start(w2e[:], w2e_d[:])
            nc.sync.dma_start(w2b[:], w2b_d[:])
            nc.sync.dma_start(b1c[:], b1c_d[:])
            nc.sync.dma_start(ident[:], ident_d[:])

            ones_row = st.tile([1, F_HID], F32)
            nc.gpsimd.memset(ones_row[:], 1.0)

            # persistent per-core state
            y1t = st.tile([F_HID, NH], F32)          # (X@W1)^T, own half
            z1te = st.tile([F_HID + 1, NH], F32)     # [Z1^T; ones], own half
            nc.gpsimd.memset(z1te[F_HID : F_HID + 1, :], 1.0)
            y1p = []                                  # [Y1,1] rows, all m
            y2all = []                                # [Y2,1] rows, all m (post-gather)
            y2b = []                                  # Y2+b2, own chunks
            for mi, (m0, mw) in enumerate(mblocks):
                y1p.append(st.tile([P, F_HID + 1], MDT, name=f"y1p{mi}"))
                y2all.append(st.tile([P, F_OUT + 1], MDT, name=f"y2a{mi}"))
            for ki, (c0, cw) in enumerate(chunks):
                y2b.append(st.tile([P, F_OUT], F32, name=f"y2b{ki}"))

            # ---------- Y1 precompute ----------
            for mi, (m0, mw) in enumerate(mblocks):
                psy = ps_m.tile([P, F_HID + 1], F32, name="psy", tag="psm")
                nc.tensor.matmul(
                    psy[:mw], xte[:, m0 : m0 + mw], w1e[:], start=True, stop=True
                )
                nc.vector.tensor_copy(y1p[mi][:mw], psy[:mw])
            for nj, nw in nblocks:
                psyt = ps_m.tile([F_HID, NBLK], F32, name="psyt", tag="psm")
                nc.tensor.matmul(
                    psyt[:, :nw],
                    w1[:],
                    xtn[:, nj : nj + nw],
                    start=True,
                    stop=True,
                )
                nc.vector.tensor_copy(y1t[:, nj : nj + nw], psyt[:, :nw])
            xpool_ctx.__exit__(None, None, None)  # free xte/xtn space
            # pools created after xtmp closes reuse its address space
            etp = es.enter_context(tc.tile_pool(name="etp", bufs=7))
            splp = es.enter_context(tc.tile_pool(name="spl", bufs=6))
            etrawp = es.enter_context(tc.tile_pool(name="etraw", bufs=4))
            sm = es.enter_context(tc.tile_pool(name="small", bufs=2))
            cachep = es.enter_context(tc.tile_pool(name="cache", bufs=1))

            # ---------- ET cache allocation (pass2 reuse) ----------
            # per nblock, cache the first (n_et_cached // len(nblocks)) mblocks
            cache_m = n_et_cached // len(nblocks)
            cache_tiles = {}
            for j, (nj, nw) in enumerate(nblocks):
                for mi in range(cache_m):
                    cache_tiles[(j, mi)] = cachep.tile(
                        [P, NBLK], MDT, name=f"etc{j}_{mi}"
                    )

            spill_tiles = {}
            for j, (nj, nw) in enumerate(nblocks):
                for mi in range(cache_m, len(mblocks)):
                    spill_tiles[(j, mi)] = dramp.tile(
                        [P, NBLK], MDT, name=f"ets{j}_{mi}"
                    )

            def scores_to_et(j, nj, nw, mi, m0, mw, write_cache):
                """scores matmul -> exp -> max(.,1) -> SBUF E^T tile [mw, nw]."""
                pss = ps_s.tile([P, NBLK], F32, name="pss")
                if rmode == 2:
                    # L = Mh Nh^T + Ml Nh^T + Mh Nl^T  (drop Ml Nl ~ 2^-22)
                    nc.tensor.matmul(
                        pss[:mw, :nw], mt_hi[:, m0 : m0 + mw],
                        mtn_hi[:, nj : nj + nw], start=True, stop=False)
                    nc.tensor.matmul(
                        pss[:mw, :nw], mt_lo[:, m0 : m0 + mw],
                        mtn_hi[:, nj : nj + nw], start=False, stop=False)
                    nc.tensor.matmul(
                        pss[:mw, :nw], mt_hi[:, m0 : m0 + mw],
                        mtn_lo[:, nj : nj + nw], start=False, stop=True)
                else:
                    nc.tensor.matmul(
                        pss[:mw, :nw], mt[:, m0 : m0 + mw],
                        mtn[:, nj : nj + nw], start=True, stop=True)
                etr = etrawp.tile([P, NBLK], F32, name="etr")
                nc.scalar.activation(
                    etr[:mw, :nw], pss[:mw, :nw], mybir.ActivationFunctionType.Exp
                )
                if write_cache:
                    et = cache_tiles[(j, mi)]
                else:
                    et = etp.tile([P, NBLK], MDT, name="et")
                nc.vector.tensor_scalar_max(et[:mw, :nw], etr[:mw, :nw], 1.0)
                return et

            # ---------- pass 1 ----------
            # Software-pipelined m-loop: emit scores (PE) PRE tiles ahead of
            # the AV matmul that consumes them, so the strict-FIFO PE queue
            # never stalls waiting on the exp/max chain of its own tile.
            PRE = 3
            nm = len(mblocks)
            for j, (nj, nw) in enumerate(nblocks):
                a1t = ps_a.tile([F_HID + 1, NBLK], F32, name="a1t", tag="acc")
                ets = {}
                for mi in range(min(PRE, nm)):
                    m0, mw = mblocks[mi]
                    ets[mi] = scores_to_et(
                        j, nj, nw, mi, m0, mw, (j, mi) in cache_tiles
                    )
                for mi, (m0, mw) in enumerate(mblocks):
                    if mi + PRE < nm:
                        m0n, mwn = mblocks[mi + PRE]
                        ets[mi + PRE] = scores_to_et(
                            j, nj, nw, mi + PRE, m0n, mwn,
                            (j, mi + PRE) in cache_tiles,
                        )
                    et1 = ets.pop(mi)
                    if (j, mi) in spill_tiles:
                        nc.sync.dma_start(
                            spill_tiles[(j, mi)][:mw, :nw], et1[:mw, :nw]
                        )
                    nc.tensor.matmul(
                        a1t[:, :nw], y1p[mi][:mw], et1[:mw, :nw],
                        start=(mi == 0), stop=(mi == nm - 1),
                    )
                # epilogue: Z1^T[:, nj:nj+nw]
                rd = sm.tile([1, NBLK], F32, name="rd")
                nc.vector.reciprocal(rd[:, :nw], a1t[F_HID : F_HID + 1, :nw])
                bcp = ps_m.tile([F_HID, NBLK], F32, name="bcp", tag="psm")
                nc.tensor.matmul(
                    bcp[:, :nw], ones_row[:], rd[:, :nw], start=True, stop=True
                )
                bcs = sm.tile([F_HID, NBLK], F32, name="bcs")
                nc.scalar.activation(
                    bcs[:, :nw], bcp[:, :nw], mybir.ActivationFunctionType.Copy
                )
                tt = sm.tile([F_HID, NBLK], F32, name="tt")
                nc.vector.tensor_tensor(
                    tt[:, :nw], a1t[:F_HID, :nw], bcs[:, :nw], mybir.AluOpType.mult
                )
                uu = sm.tile([F_HID, NBLK], F32, name="uu")
                nc.vector.tensor_tensor(
                    uu[:, :nw], tt[:, :nw], y1t[:, nj : nj + nw], mybir.AluOpType.add
                )
                nc.scalar.activation(
                    z1te[:F_HID, nj : nj + nw],
                    uu[:, :nw],
                    mybir.ActivationFunctionType.Relu,
                    bias=b1c[:],
                )

            # ---------- Y2 (own half) + AllGather ----------
            cc_in = dramp.tile([NH, F_OUT + 1], MDT)
            cc_out = dramp.tile([N, F_OUT + 1], MDT)
            for ki, (c0, cw) in enumerate(chunks):
                psy2 = ps_m.tile([P, F_OUT + 1], F32, name="psy2", tag="psm")
                nc.tensor.matmul(
                    psy2[:cw], z1te[:, c0 : c0 + cw], w2e[:], start=True, stop=True
                )
                y2own = sm.tile([P, F_OUT + 1], MDT, name="y2own")
                nc.vector.tensor_copy(y2own[:cw], psy2[:cw])
                nc.sync.dma_start(cc_in[c0 : c0 + cw, :], y2own[:cw])
                psy2b = ps_m.tile([P, F_OUT], F32, name="psy2b", tag="psm")
                nc.tensor.matmul(
                    psy2b[:cw], z1te[:, c0 : c0 + cw], w2b[:], start=True, stop=True
                )
                nc.vector.tensor_copy(y2b[ki][:cw], psy2b[:cw])
            nc.gpsimd.collective_compute(
                "AllGather",
                mybir.AluOpType.bypass,
                replica_groups=[[0, 1], [2, 3], [4, 5], [6, 7]],
                ins=[cc_in.opt()],
                outs=[cc_out.opt()],
            )
            for mi, (m0, mw) in enumerate(mblocks):
                nc.sync.dma_start(y2all[mi][:mw], cc_out[m0 : m0 + mw, :])

            # ---------- pass 2 ----------
            # No scores here at all: cached tiles come from SBUF, the rest
            # were spilled to DRAM during pass 1 and are DMA-prefetched.
            for j, (nj, nw) in enumerate(nblocks):
                a2t = ps_a.tile([F_OUT + 1, NBLK], F32, name="a2t", tag="acc")
                loads = {}
                for mi in range(cache_m, nm):
                    m0, mw = mblocks[mi]
                    lt = splp.tile([P, NBLK], MDT, name="spl")
                    nc.sync.dma_start(
                        lt[:mw, :nw], spill_tiles[(j, mi)][:mw, :nw]
                    )
                    loads[mi] = lt
                for mi, (m0, mw) in enumerate(mblocks):
                    if (j, mi) in cache_tiles:
                        et = cache_tiles[(j, mi)]
                    else:
                        et = loads.pop(mi)
                    nc.tensor.matmul(
                        a2t[:, :nw], y2all[mi][:mw], et[:mw, :nw],
                        start=(mi == 0), stop=(mi == nm - 1),
                    )
                # epilogue per 128-chunk: transpose -> divide -> combine -> store
                a2s = sm.tile([F_OUT + 1, NBLK], F32, name="a2s")
                nc.vector.tensor_copy(a2s[:, :nw], a2t[:, :nw])
                for c0, cw in _blocks(nw, P):
                    g0 = nj + c0  # offset within own half
                    ki = g0 // P
                    trp = ps_m.tile([P, F_OUT + 1], F32, name="trp", tag="psm")
                    nc.tensor.transpose(
                        trp[:cw],
                        a2s[:, c0 : c0 + cw],
                        ident[: F_OUT + 1, : F_OUT + 1],
                    )
                    rd2 = sm.tile([P, 1], F32, name="rd2")
                    nc.vector.reciprocal(rd2[:cw], trp[:cw, F_OUT : F_OUT + 1])
                    vv = sm.tile([P, F_OUT], F32, name="vv")
                    nc.vector.tensor_scalar(
                        vv[:cw],
                        trp[:cw, :F_OUT],
                        rd2[:cw],
                        None,
                        op0=mybir.AluOpType.mult,
                    )
                    z2 = sm.tile([P, F_OUT], F32, name="z2")
                    nc.vector.tensor_tensor(
                        z2[:cw], y2b[ki][:cw], vv[:cw], mybir.AluOpType.subtract
                    )
                    nc.sync.dma_start(out_d[g0 : g0 + cw, :], z2[:cw])

    nc.compile()
    return nc


def _tf32_round(x):
    xi = x.astype(np.float32).view(np.uint32)
    xi = (xi + np.uint32(0x1000)) & np.uint32(0xFFFFE000)
    return xi.view(np.float32)


def _prep_inputs(X, M, W1, b1, W2, b2, rmode=0):
    """Host-side prep: transposes + affine-trick weight matrices."""
    X = np.ascontiguousarray(X, dtype=np.float32)
    M = np.ascontiguousarray(M, dtype=np.float32)
    W1 = np.asarray(W1, dtype=np.float32)
    b1 = np.asarray(b1, dtype=np.float32)
    W2 = np.asarray(W2, dtype=np.float32)
    b2 = np.asarray(b2, dtype=np.float32)

    MT = np.ascontiguousarray(M.transpose(0, 2, 1))  # [B,16,N]
    XT = X.transpose(0, 2, 1)  # [B,32,N]
    XTe = np.concatenate(
        [XT, np.ones((B, 1, N), np.float32)], axis=1
    )  # [B,33,N]
    XTe = np.ascontiguousarray(XTe)

    W1e = np.zeros((F_IN + 1, F_HID + 1), np.float32)
    W1e[:F_IN, :F_HID] = W1
    W1e[F_IN, F_HID] = 1.0  # ones column of [Y1,1] via the ones row of XTe

    W2e = np.zeros((F_HID + 1, F_OUT + 1), np.float32)
    W2e[:F_HID, :F_OUT] = W2
    W2e[F_HID, F_OUT] = 1.0

    W2b = np.zeros((F_HID + 1, F_OUT), np.float32)
    W2b[:F_HID] = W2
    W2b[F_HID] = b2

    b1c = np.ascontiguousarray(b1.reshape(F_HID, 1))
    ident = np.eye(P, dtype=np.float32)

    if rmode == 2:
        MT_hi = _tf32_round(MT)
        MT_lo = _tf32_round(MT - MT_hi)

    in_maps = []
    for c in range(NCORES):
        b, h = c // 2, c % 2
        n0 = h * NH
        m = {
            "xte": XTe[b],
            "xtn": np.ascontiguousarray(XT[b][:, n0 : n0 + NH]),
            "w1": W1,
            "w1e": W1e,
            "w2e": W2e,
            "w2b": W2b,
            "b1c": b1c,
            "ident": ident,
        }
        if rmode == 2:
            m["mt_hi"] = MT_hi[b]
            m["mt_lo"] = MT_lo[b]
            m["mtn_hi"] = np.ascontiguousarray(MT_hi[b][:, n0 : n0 + NH])
            m["mtn_lo"] = np.ascontiguousarray(MT_lo[b][:, n0 : n0 + NH])
        else:
            m["mt"] = MT[b]
            m["mtn"] = np.ascontiguousarray(MT[b][:, n0 : n0 + NH])
        in_maps.append(m)
    return in_maps


_NC_CACHE = {}


def _get_nc(n_et_cached=36, rmode=0):
    key = (n_et_cached, rmode)
    if key not in _NC_CACHE:
        _NC_CACHE[key] = build_nc(n_et_cached=n_et_cached, rmode=rmode)
    return _NC_CACHE[key]


def kernel(X, M, W1, b1, W2, b2, _trace=False, _n_et_cached=36, _rmode=0):
    nc = _get_nc(_n_et_cached, _rmode)
    in_maps = _prep_inputs(X, M, W1, b1, W2, b2, rmode=_rmode)
    res = run_bass_kernel_spmd(
        nc, in_maps, core_ids=list(range(NCORES)), trace=_trace
    )
    out = np.empty((B, N, F_OUT), np.float32)
    for c in range(NCORES):
        b, h = c // 2, c % 2
        out[b, h * NH : (h + 1) * NH] = res.results[c]["out"]
    kernel.last_results = res
    return out
